# revision 7
# baseline (speedup 1.0000x reference)
"""Trainium2 Bass kernel for nn_ContinuousPositionBias (embedding_lookup).

Key idea: rpi has 2-level Toeplitz structure
    rpi[(ih,iw),(jh,jw)] = (ih-jh+23)*47 + (iw-jw+23)
so the per-pair gather out[b,h,1+i,1+j] = bias[b, rpi[i,j], h] never needs a
real gather.  Feeding the CPB MLP with the *reversed* coords table gives
u[k] = bias[2208-k] and then
    out[b,h,1+24*ih+iw,1+24*jh+jw] = u[b, 47*(jh-ih+23) + (jw-iw+23), h].

Per sample we build a "strip" in SBUF laid out [16 heads (partitions),
24 iw-lines x 1128] with
    strip[h, iw*1128 + d*24 + jw] = u[h, 47d + jw + 23 - iw]
so each output row (b,h,1+24*ih+iw, 1:) is a 576-element contiguous window
of the iw-line starting at (23-ih)*24.  The strip is built by the compute
engines (DVE/ACT/Pool tensor copies with a 3-level shifted-window access
pattern shared across head partitions) so the DMA engines spend their time
exclusively on the unavoidable 85MB/core of output writes, which stream at
full bandwidth as 2304B-contiguous descriptors.  Prefix row/col zeros are
broadcast writes folded into large-descriptor DMAs.

Sharding: batch (bs=32) split across 8 cores, 4 samples each.  MLP params and
the coords table are tiny and replicated (baked into per-core inputs).
"""
import sys

sys.path.insert(0, "/opt/trn_rl_repo")

import numpy as np

import concourse.bass as bass
import concourse.bacc as bacc
import concourse.mybir as mybir
from concourse.tile import TileContext
from concourse import bass_utils

# problem constants (fixed by the nn.Module definition)
WH = WW = 24
N = WH * WW                # 576
D47 = 2 * WH - 1           # 47
T = D47 * D47              # 2209
H = 16                     # num heads
RPB = 512                  # MLP hidden dim
BS = 32
NCORES = 8
BSL = BS // NCORES         # 4 samples per core
P1 = 1                     # num_prefix_tokens
NO = N + P1                # 577
LW = D47 * WH              # strip line width per (head, iw): 1128
OUT_HW = NO * NO           # 332929
OUT_B = H * OUT_HW         # per-sample output elements
TP = 2212                  # token pitch per sample, padded so every fp32r
                           # matmul chunk is >=256 and a multiple of 4
SLW = WH * LW              # full strip free size per head: 27072

_CACHE = {}


def _build_program():
    """Build the per-core Bass program (identical for all 8 cores)."""
    nc = bacc.Bacc(
        "TRN2",
        target_bir_lowering=False,
        debug=False,
        enable_asserts=False,
        num_devices=NCORES,
    )
    f32 = mybir.dt.float32
    f32r = mybir.dt.float32r

    xt = nc.dram_tensor("xt", (3, BSL * TP), f32r, kind="ExternalInput")
    w1a = nc.dram_tensor("w1a", (128, RPB), f32r, kind="ExternalInput")
    w2r = nc.dram_tensor("w2r", (128, 4 * H), f32r, kind="ExternalInput")
    out = nc.dram_tensor("out", (BSL, H, NO, NO), f32, kind="ExternalOutput")

    chunks = [(0, 512), (512, 512), (1024, 512), (1536, 420), (1956, 256)]

    IWR = 8                  # iw-lines per drain range
    NR = WH // IWR           # 3 drain ranges per sample
    # iw-split of each range across the three copy engines (DVE/ACT/Pool);
    # DVE fp32 SBUF-SBUF copies run in 2x mode so it takes the biggest share
    ESPLIT = [("vector", 0, 4), ("scalar", 4, 3), ("gpsimd", 7, 1)]

    with TileContext(nc) as tc:
        with (
            tc.tile_pool(name="singles", bufs=1) as singles,
            tc.tile_pool(name="htpool", bufs=2) as htpool,
            tc.tile_pool(name="psum_h", bufs=2, space="PSUM") as psum_hp,
            tc.tile_pool(name="psum_u", bufs=2, space="PSUM") as psum_up,
        ):
            # critical-path loads FIRST so the MLP can start ASAP.  The host
            # sends xt/w1a pre-replicated x4 so each lands in the PE row
            # groups 0/32/64/96 with a single DMA (2-level partition step).
            xt_s = singles.tile([128, BSL * TP], f32r)
            for dt in range(4):
                nc.sync.dma_start(xt_s[32 * dt:32 * dt + 3, :], xt[:])
            w1_s = singles.tile([128, RPB], f32r)
            for dt in range(4):
                nc.sync.dma_start(w1_s[32 * dt:32 * dt + 3, :], w1a[:3, :])
            w2_s = singles.tile([128, 4 * H], f32r)
            nc.sync.dma_start(w2_s[:], w2r[:])
            u_s = singles.tile([H, BSL * TP], f32)
            strip = singles.tile([H, SLW], f32)

            # Row-0 / column-0 zero padding is NOT written by the kernel:
            # run_bass_kernel_spmd (and its bass2jax axon redirect) hands the
            # NEFF zero-initialized ExternalOutput buffers — the documented
            # contract for kernels that don't write every element.

            def emit_mm1(b, ci):
                c0, n = chunks[ci]
                phs = []
                for dt in range(4):
                    ph = psum_hp.tile([128, 512], f32, tag="ph", name="ph")
                    nc.tensor.matmul(
                        ph[:, :n],
                        w1_s[32 * dt:32 * dt + 3, dt * 128:(dt + 1) * 128],
                        xt_s[32 * dt:32 * dt + 3, b * TP + c0: b * TP + c0 + n],
                        start=True,
                        stop=True,
                        tile_position=(32 * dt, 0),
                    )
                    phs.append(ph)
                # relu split ACT/DVE to halve the per-chunk relu chain
                hts = []
                for dt in range(4):
                    ht = htpool.tile([128, 512], f32r, tag=f"ht{dt}",
                                     name=f"ht{dt}")
                    if dt < 2:
                        nc.scalar.activation(
                            ht[:, :n],
                            phs[dt][:, :n],
                            mybir.ActivationFunctionType.Relu,
                        )
                    else:
                        nc.vector.tensor_scalar_max(
                            ht[:, :n], phs[dt][:, :n], 0.0
                        )
                    hts.append(ht)
                return hts

            def emit_mm2(b, ci, hts):
                c0, n = chunks[ci]
                pu = psum_up.tile([H, 512], f32)
                for dt in range(4):
                    nc.tensor.matmul(
                        pu[:, :n],
                        w2_s[:, dt * H:(dt + 1) * H],
                        hts[dt][:, :n],
                        start=(dt == 0),
                        stop=(dt == 3),
                    )
                nc.vector.tensor_copy(u_s[:, b * TP + c0: b * TP + c0 + n], pu[:, :n])

            def emit_mlp(b):
                # software-pipelined: mm1(c+1) is emitted before mm2(c) so
                # the in-order PE queue never stalls waiting for a relu.
                hts_prev = None
                for ci in range(len(chunks)):
                    hts = emit_mm1(b, ci)
                    if hts_prev is not None:
                        emit_mm2(b, ci - 1, hts_prev)
                    hts_prev = hts
                emit_mm2(b, len(chunks) - 1, hts_prev)

            def emit_build(b, r):
                # strip[h, iw*LW + d*24 + jw] = u[h, b*TP + 47d + jw + 23-iw]
                # One copy per engine, iw-split; the shifted-window source AP
                # [[-1,niw],[47,47],[1,24]] is shared across head partitions.
                iw_base = r * IWR
                for eng, e0, niw in ESPLIT:
                    iw0 = iw_base + e0
                    src = bass.AP(
                        tensor=u_s[:].tensor,
                        offset=u_s[:].offset + b * TP + (WH - 1) - iw0,
                        ap=[[BSL * TP, H], [-1, niw], [D47, D47], [1, WH]],
                    )
                    dst = bass.AP(
                        tensor=strip[:].tensor,
                        offset=strip[:].offset + iw0 * LW,
                        ap=[[SLW, H], [LW, niw], [WH, D47], [1, WH]],
                    )
                    if eng == "vector":
                        nc.vector.tensor_copy(dst, src)
                    elif eng == "scalar":
                        nc.scalar.copy(dst, src)
                    else:
                        nc.gpsimd.tensor_copy(dst, src)

            def emit_drain(b, r):
                # DMA APs are limited to 3 dims and dim0 must step SBUF
                # partitions, so one DMA per (sample, iw-range, ih window):
                # 16 heads x IWR iw-lines x 576 contiguous elements.
                iw0 = r * IWR
                for ih in reversed(range(WH)):
                    osrc = bass.AP(
                        tensor=strip[:].tensor,
                        offset=strip[:].offset + iw0 * LW
                        + (WH - 1 - ih) * WH,
                        ap=[[SLW, H], [LW, IWR], [1, N]],
                    )
                    odst = bass.AP(
                        tensor=out[:].tensor,
                        offset=b * OUT_B + (1 + ih * WH + iw0) * NO + 1,
                        ap=[[OUT_HW, H], [NO, IWR], [1, N]],
                    )
                    nc.sync.dma_start(odst, osrc)

            emit_mlp(0)
            for b in range(BSL):
                for r in range(NR):
                    emit_build(b, r)
                    emit_drain(b, r)
                    if r == 0 and b + 1 < BSL:
                        emit_mlp(b + 1)

    nc.compile()
    return nc


def _host_prep(glob_pos, coords_table, W1, b1, W2):
    f32 = np.float32
    g = np.asarray(glob_pos, f32)[0]            # (32, 4)
    pos = g[..., 2:] / g[..., :2] * f32(8.0)
    pos = np.sign(pos) * np.log2(np.abs(pos) + f32(1.0)) / f32(3.0)
    pos = pos * f32(2.0) - f32(1.0)             # (32, 2)

    ct_rev = np.asarray(coords_table, f32)[::-1]  # (T, 2)
    W1 = np.asarray(W1, f32)
    b1 = np.asarray(b1, f32)
    W2 = np.asarray(W2, f32)

    # [W1; b1] packed rows; the device replicates into PE row groups 0/32/64/96
    w1a = np.zeros((128, RPB), f32)
    w1a[:3] = np.concatenate([W1, b1[None, :]], axis=0)
    w2r = np.empty((128, 4 * H), f32)
    for dt in range(4):
        w2r[:, dt * H:(dt + 1) * H] = W2[dt * 128:(dt + 1) * 128]

    in_maps = []
    for core in range(NCORES):
        xt3 = np.ones((3, BSL * TP), f32)
        for bl in range(BSL):
            xt3[:2, bl * TP:bl * TP + T] = (ct_rev + pos[core * BSL + bl]).T
        in_maps.append({"xt": xt3, "w1a": w1a, "w2r": w2r})
    return in_maps


def kernel(glob_pos, coords_table, rpi, W1, b1, W2, num_prefix_tokens,
           _trace=False):
    assert int(num_prefix_tokens) == P1
    if "nc" not in _CACHE:
        _CACHE["nc"] = _build_program()
    nc = _CACHE["nc"]

    in_maps = _host_prep(glob_pos, coords_table, W1, b1, W2)
    try:
        res = bass_utils.run_bass_kernel_spmd(
            nc, in_maps, core_ids=list(range(NCORES)), trace=_trace
        )
    except ModuleNotFoundError:
        # axon NTFF profiling hook unavailable in this container
        res = bass_utils.run_bass_kernel_spmd(
            nc, in_maps, core_ids=list(range(NCORES)), trace=False
        )
    _CACHE["last"] = res
    out = np.concatenate([r["out"] for r in res.results], axis=0)
    return out
